# revision 1
# baseline (speedup 1.0000x reference)
"""DGCNN (3x DynamicEdgeConv + point MLP) Trainium2 kernel.

Self-contained: builds a Bass/Tile program that processes 2 point clouds per
NeuronCore and runs it SPMD on 8 cores (data-parallel over the batch of 16).

Algorithm per cloud, per edge-conv layer (feature-major layout XT [C, N]):
  S = 2*X@X.T - |x_j|^2          (row-shifted -distance; same per-row top-k)
  top-30 neighbor selection on DVE:
     pack local column index (8 bits, complemented) into S's mantissa LSBs,
     screen top-16 per 256-chunk with Max8/MatchReplace, 4 merge rounds with
     MaxIndex position recovery (chunk id from candidate position).
  gather neighbor/self features via GPSIMD indirect_copy (per-core wrapped
  index lists), edge MLP via TensorE (h1 = relu(W@[xj;xi]+b0), h2 = w1@h1),
  max-aggregate over the 30 edges via strided tensor_reduce.
Then the pointwise MLP 192->1024->256->128->3 on TensorE.
"""
import sys
import numpy as np

for _p in ("/opt/trn_rl_repo", "/root/.axon_site/_ro/trn_rl_repo"):
    if _p not in sys.path:
        sys.path.append(_p)

B, N, KNN = 16, 2048, 30
NCORES = 8
CPC = B // NCORES          # clouds per core
P = 128                    # partitions
NT = N // P                # row tiles per cloud (16)
CHUNK = 256                # selection screen chunk
NCH = N // CHUNK           # 8 chunks
DEPTH = 16                 # per-chunk screen depth
NB = 8                     # packed local-index bits
KSEL = 32                  # extracted per row (use first 30)
SUB = 16                   # points per edge sub-chunk
ESUB = SUB * KNN           # 480 edge slots per sub-chunk
NSUB = P // SUB            # 8 sub-chunks per tile
NEG = -3.0e38

_PROGRAM_CACHE = {}


def _build_program():
    import concourse.bass as bass
    import concourse.bacc as bacc
    import concourse.tile as tile
    from concourse import mybir
    from contextlib import ExitStack

    f32 = mybir.dt.float32
    u32 = mybir.dt.uint32
    u16 = mybir.dt.uint16
    Alu = mybir.AluOpType
    Act = mybir.ActivationFunctionType

    nc = bacc.Bacc()

    # ---------------- DRAM parameters ----------------
    def din(name, shape, dtype=f32):
        return nc.declare_dram_parameter(name, list(shape), dtype, isOutput=False)

    posT_d = din("posT", [CPC, 3, N])
    iota_d = din("iota_lc", [P, N], u32)
    iwrap_d = din("iwrap", [NT, 16, 8 * KNN], u16)
    conv_w = []
    for l, c in ((1, 3), (2, 64), (3, 64)):
        conv_w.append(dict(
            C=c,
            w0a=din(f"c{l}_w0a", [c, 64]),
            w0b=din(f"c{l}_w0b", [c, 64]),
            w1=din(f"c{l}_w1", [64, 64]),
            b0=din(f"c{l}_b0", [64, 1]),
            b1=din(f"c{l}_b1", [64, 1]),
        ))
    mw0k0_d = din("mlp_w0k0", [128, 1024])
    mw0k1_d = din("mlp_w0k1", [64, 1024])
    mb0_d = din("mlp_b0", [128, 8])
    mw1_d = din("mlp_w1r", [128, 8, 256])
    mb1_d = din("mlp_b1", [128, 2])
    mw2_d = din("mlp_w2r", [128, 2, 128])
    mb2_d = din("mlp_b2", [128, 1])
    finw_d = din("fin_w", [128, 3])
    finb_d = din("fin_brep", [128, 3])
    out_d = nc.declare_dram_parameter("out", [CPC, N, 3], f32, isOutput=True)

    with tile.TileContext(nc) as tc, ExitStack() as CTX:
        # ------------- persistent pools -------------
        persist = CTX.enter_context(tc.tile_pool(name="persist", bufs=1))

        iota_lc = persist.tile([P, N], u32)
        nc.sync.dma_start(iota_lc, iota_d[:])

        # per-layer H1pre weights (built on device), w1, biases
        wh1 = []
        for l in range(3):
            cw = conv_w[l]
            C = cw["C"]
            w0a = persist.tile([C, 64], f32, name=f"w0a_{l}")
            nc.sync.dma_start(w0a, cw["w0a"][:])
            w0b = persist.tile([C, 64], f32, name=f"w0b_{l}")
            nc.sync.dma_start(w0b, cw["w0b"][:])
            w0diff = persist.tile([C, 64], f32, name=f"w0diff_{l}")
            nc.vector.tensor_sub(w0diff, w0a, w0b)
            kdim = 32 if l == 0 else 128
            wh = persist.tile([kdim, 64], f32, name=f"wh1_{l}")
            nc.vector.memset(wh, 0.0)
            if l == 0:
                nc.sync.dma_start(wh[0:3, :], w0b)
                nc.sync.dma_start(wh[16:19, :], w0diff)
            else:
                for g in range(4):
                    nc.sync.dma_start(wh[32 * g:32 * g + 16, :], w0b[16 * g:16 * g + 16, :])
                    nc.sync.dma_start(wh[32 * g + 16:32 * g + 32, :], w0diff[16 * g:16 * g + 16, :])
            w1t = persist.tile([64, 64], f32, name=f"w1_{l}")
            nc.sync.dma_start(w1t, cw["w1"][:])
            b0t = persist.tile([64, 1], f32, name=f"b0_{l}")
            nc.sync.dma_start(b0t, cw["b0"][:])
            b1t = persist.tile([64, 1], f32, name=f"b1_{l}")
            nc.sync.dma_start(b1t, cw["b1"][:])
            wh1.append(dict(C=C, wh=wh, kdim=kdim, w1=w1t, b0=b0t, b1=b1t))

        # MLP weights
        mw0k0 = persist.tile([128, 1024], f32)
        nc.sync.dma_start(mw0k0, mw0k0_d[:])
        mw0k1 = persist.tile([64, 1024], f32)
        nc.sync.dma_start(mw0k1, mw0k1_d[:])
        mb0 = persist.tile([128, 8], f32)
        nc.sync.dma_start(mb0, mb0_d[:])
        mw1 = persist.tile([128, 8, 256], f32)
        nc.sync.dma_start(mw1, mw1_d[:])
        mb1 = persist.tile([128, 2], f32)
        nc.sync.dma_start(mb1, mb1_d[:])
        mw2 = persist.tile([128, 2, 128], f32)
        nc.sync.dma_start(mw2, mw2_d[:])
        mb2 = persist.tile([128, 1], f32)
        nc.sync.dma_start(mb2, mb2_d[:])
        finw = persist.tile([128, 3], f32)
        nc.sync.dma_start(finw, finw_d[:])
        finb = persist.tile([128, 3], f32)
        nc.sync.dma_start(finb, finb_d[:])

        ones_col = persist.tile([64, 1], f32)
        nc.vector.memset(ones_col, 1.0)

        # ---------------- per-cloud processing ----------------
        for cloud in range(CPC):
            with ExitStack() as cctx:
                cloudp = cctx.enter_context(tc.tile_pool(name=f"cloud{cloud}", bufs=1))

                # per-tile gather index tensors (i-rows persist across layers)
                idx_tiles = []
                for t in range(NT):
                    it = cloudp.tile([P, 8 * KNN], u16, name=f"idxt{t}", tag="idxt", bufs=NT)
                    for rep in range(4):
                        nc.sync.dma_start(it[32 * rep + 16:32 * rep + 32, :], iwrap_d[t])
                    idx_tiles.append(it)

                # feature tensors (aug: +1 ones row used as S-matmul lhsT rows)
                xt1 = cloudp.tile([33, N], f32)     # layer1 input (pos), row32 = ones
                nc.vector.memset(xt1, 0.0)
                nc.sync.dma_start(xt1[0:3, :], posT_d[cloud])
                nc.vector.memset(xt1[32:33, :], 1.0)
                xt2 = cloudp.tile([65, N], f32)     # x1 + ones row
                nc.vector.memset(xt2[64:65, :], 1.0)
                xt3 = cloudp.tile([65, N], f32)     # x2 + ones row
                nc.vector.memset(xt3[64:65, :], 1.0)
                x12 = cloudp.tile([128, N], f32)    # [x1; x2] for MLP
                x3 = cloudp.tile([64, N], f32)      # x3 for MLP
                xtaugs = [xt1, xt2, xt3]

                for l in range(3):
                    with ExitStack() as lctx:
                        cw = wh1[l]
                        C = cw["C"]
                        augrow = 32 if l == 0 else 64   # partition of the ones/x2 row
                        caug = augrow + 1
                        xtaug = xtaugs[l]
                        xt_next = None if l == 2 else xtaugs[l + 1]

                        lp = lctx.enter_context(tc.tile_pool(name=f"lay{cloud}_{l}", bufs=1))
                        work = lctx.enter_context(tc.tile_pool(name=f"lw{cloud}_{l}", bufs=2))
                        psel = lctx.enter_context(tc.tile_pool(name=f"ps{cloud}_{l}", bufs=2))
                        spsum_p = lctx.enter_context(tc.tile_pool(name=f"sp{cloud}_{l}", bufs=1, space="PSUM"))
                        edge_ps = lctx.enter_context(tc.tile_pool(name=f"ep{cloud}_{l}", bufs=2, space="PSUM"))

                        # ---- R = [2*XT ; -x2] ----
                        R = lp.tile([caug, N], f32)
                        if l == 0:
                            nc.vector.memset(R, 0.0)
                        nc.scalar.activation(R[0:C, :], xtaug[0:C, :], Act.Copy, scale=2.0)
                        sq = lp.tile([C, N], f32)
                        nc.scalar.activation(sq, xtaug[0:C, :], Act.Square)
                        for nchk in range(4):
                            x2ps = spsum_p.tile([1, 512], f32, name=f"x2ps{cloud}_{l}_{nchk}",
                                                tag="spsum")
                            nc.tensor.matmul(x2ps,
                                             lhsT=ones_col[0:C, :],
                                             rhs=sq[:, nchk * 512:(nchk + 1) * 512],
                                             start=True, stop=True)
                            nc.scalar.activation(R[augrow:caug, nchk * 512:(nchk + 1) * 512],
                                                 x2ps, Act.Copy, scale=-1.0)

                        # ---- gather data D [128, N] (interleaved j/i copies) ----
                        D = lp.tile([P, N], f32)
                        if l == 0:
                            nc.vector.memset(D, 0.0)
                            nc.sync.dma_start(D[0:3, :], xtaug[0:3, :])
                            nc.sync.dma_start(D[16:19, :], xtaug[0:3, :])
                        else:
                            for g in range(4):
                                nc.sync.dma_start(D[32 * g:32 * g + 16, :], xtaug[16 * g:16 * g + 16, :])
                                nc.sync.dma_start(D[32 * g + 16:32 * g + 32, :], xtaug[16 * g:16 * g + 16, :])

                        # ---- 3-stage software pipeline over the 16 row tiles:
                        #   A(t): S matmul + PSUM->SBUF copy
                        #   B(t): selection + wrapped-idx DMAs
                        #   C(t): gather + edge MLP + aggregation
                        # Skewed emission keeps every engine's in-order stream
                        # supplied with ready work (2-tile lookahead).
                        scp_tiles = {}

                        def stage_a(t):
                            spsum = spsum_p.tile([P, N], f32, name=f"spsum{cloud}_{l}_{t}", tag="spsum")
                            lhsT = xtaug[:, t * P:(t + 1) * P]
                            for nchk in range(4):
                                nc.tensor.matmul(spsum[:, nchk * 512:(nchk + 1) * 512],
                                                 lhsT=lhsT,
                                                 rhs=R[:, nchk * 512:(nchk + 1) * 512],
                                                 start=True, stop=True)
                            scp = work.tile([P, N], f32, tag="spk", name=f"scp{t}", bufs=3)
                            nc.scalar.activation(scp, spsum, Act.Copy)
                            scp_tiles[t] = scp

                        def stage_b(t):
                            # selection: exact values; indices via full-row max_index;
                            # first-match semantics == top_k's smallest-index tie-break
                            scp = scp_tiles[t]
                            cand = psel.tile([P, NCH * DEPTH], f32, tag="cand")
                            scr = psel.tile([P, CHUNK], f32, tag="scr")
                            for c in range(NCH):
                                chunk = scp[:, c * CHUNK:(c + 1) * CHUNK]
                                nc.vector.max(out=cand[:, c * DEPTH:c * DEPTH + 8], in_=chunk)
                                nc.vector.match_replace(out=scr, in_to_replace=cand[:, c * DEPTH:c * DEPTH + 8],
                                                        in_values=chunk, imm_value=NEG)
                                nc.vector.max(out=cand[:, c * DEPTH + 8:c * DEPTH + 16], in_=scr)
                            topv = psel.tile([P, KSEL], f32, tag="topv")
                            idx_sel = psel.tile([P, KSEL], u16, tag="idx_sel")
                            cscr = psel.tile([P, NCH * DEPTH], f32, tag="cscr")
                            cur = cand
                            for r in range(KSEL // 8):
                                tv = topv[:, r * 8:(r + 1) * 8]
                                nc.vector.max(out=tv, in_=cur)
                                nc.vector.max_index(out=idx_sel[:, r * 8:(r + 1) * 8],
                                                    in_max=tv, in_values=scp)
                                if r < KSEL // 8 - 1:
                                    nxt = cscr if cur is cand else cand
                                    nc.vector.match_replace(out=nxt, in_to_replace=tv,
                                                            in_values=cur, imm_value=NEG)
                                    cur = nxt

                            # wrapped j-idx build (8 + 3 DMAs)
                            it = idx_tiles[t]
                            for q in range(8):
                                nc.sync.dma_start(it[0:16, KNN * q:KNN * (q + 1)],
                                                  idx_sel[16 * q:16 * (q + 1), 0:KNN])
                            for rep in range(1, 4):
                                nc.sync.dma_start(it[32 * rep:32 * rep + 16, :], it[0:16, :])

                        def stage_c(t):
                            it = idx_tiles[t]
                            # gather (dst limited to 1024 elem/partition per inst)
                            G = work.tile([P, P * KNN], f32, tag="G")
                            for g in range(4):
                                nc.gpsimd.indirect_copy(
                                    out=G[:, 960 * g:960 * (g + 1)], data=D,
                                    idxs=it[:, 60 * g:60 * (g + 1)],
                                    i_know_ap_gather_is_preferred=True)

                            # edge MLP + aggregate per sub-chunk
                            for q in range(NSUB):
                                gsl = G[:, q * ESUB:(q + 1) * ESUB]
                                h1p = edge_ps.tile([64, ESUB], f32, tag="h1p")
                                nc.tensor.matmul(h1p, lhsT=cw["wh"],
                                                 rhs=gsl[0:cw["kdim"], :],
                                                 start=True, stop=True)
                                h1 = work.tile([64, ESUB], f32, tag="h1")
                                nc.scalar.activation(h1, h1p, Act.Relu, bias=cw["b0"])
                                h2p = edge_ps.tile([64, ESUB], f32, tag="h2p")
                                nc.tensor.matmul(h2p, lhsT=cw["w1"], rhs=h1,
                                                 start=True, stop=True)
                                # max over the 30 edges of each point: layout [64, 30k, 16r]
                                h2v = h2p.rearrange("p (k r) -> p r k", r=16)
                                colsl = slice(t * P + q * SUB, t * P + (q + 1) * SUB)
                                red = work.tile([64, SUB], f32, tag="red")
                                nc.vector.tensor_reduce(out=red, in_=h2v,
                                                        axis=mybir.AxisListType.X,
                                                        op=Alu.max)
                                xdst = x3 if l == 2 else xt_next
                                nc.vector.tensor_scalar_add(xdst[0:64, colsl], red, cw["b1"])

                        for k in range(NT + 2):
                            if k < NT:
                                stage_a(k)
                            if 1 <= k <= NT:
                                stage_b(k - 1)
                            if k >= 2:
                                stage_c(k - 2)

                        # copy x_out into MLP input stack
                        if l == 0:
                            nc.sync.dma_start(x12[0:64, :], xt2[0:64, :])
                        elif l == 1:
                            nc.sync.dma_start(x12[64:128, :], xt3[0:64, :])

                # ---------------- pointwise MLP ----------------
                with ExitStack() as mctx:
                    mp = mctx.enter_context(tc.tile_pool(name=f"mlp{cloud}", bufs=2))
                    mps = mctx.enter_context(tc.tile_pool(name=f"mlpp{cloud}", bufs=4, space="PSUM"))
                    NCHK = 512
                    for nchk in range(N // NCHK):
                        csl = slice(nchk * NCHK, (nchk + 1) * NCHK)
                        h1m = mp.tile([128, 8, NCHK], f32, tag="h1m")
                        for m in range(8):
                            msl = slice(m * 128, (m + 1) * 128)
                            hp = mps.tile([128, NCHK], f32, tag="hp")
                            nc.tensor.matmul(hp, lhsT=mw0k0[:, msl], rhs=x12[:, csl],
                                             start=True, stop=False)
                            nc.tensor.matmul(hp, lhsT=mw0k1[:, msl], rhs=x3[:, csl],
                                             start=False, stop=True)
                            nc.scalar.activation(h1m[:, m, :], hp, Act.Relu, bias=mb0[:, m:m + 1])
                        h2m = mp.tile([128, 2, NCHK], f32, tag="h2m")
                        for m in range(2):
                            hp = mps.tile([128, NCHK], f32, tag="hp")
                            for s in range(8):
                                nc.tensor.matmul(hp, lhsT=mw1[:, s, m * 128:(m + 1) * 128],
                                                 rhs=h1m[:, s, :],
                                                 start=(s == 0), stop=(s == 7))
                            nc.scalar.activation(h2m[:, m, :], hp, Act.Relu, bias=mb1[:, m:m + 1])
                        hp3 = mps.tile([128, NCHK], f32, tag="hp")
                        for s in range(2):
                            nc.tensor.matmul(hp3, lhsT=mw2[:, s, :], rhs=h2m[:, s, :],
                                             start=(s == 0), stop=(s == 1))
                        h3m = mp.tile([128, NCHK], f32, tag="h3m")
                        nc.vector.tensor_scalar_add(h3m, hp3, mb2)
                        # final: out[pt, 3] = h3m[:, ptchunk].T @ finw + finb
                        for pchk in range(NCHK // 128):
                            fp = mps.tile([128, 3], f32, tag="fp")
                            nc.tensor.matmul(fp, lhsT=h3m[:, pchk * 128:(pchk + 1) * 128],
                                             rhs=finw, start=True, stop=True)
                            fo = mp.tile([128, 3], f32, tag="fo")
                            nc.vector.tensor_tensor(out=fo, in0=fp, in1=finb, op=Alu.add)
                            nc.sync.dma_start(
                                out_d[cloud, nchk * NCHK + pchk * 128:nchk * NCHK + (pchk + 1) * 128, :],
                                fo)
    nc.compile()
    return nc


def _host_inputs(inputs):
    """Build the per-core input maps (pure layout/indexing work)."""
    pos = np.ascontiguousarray(inputs["pos"], np.float32)

    iota_lc = np.broadcast_to(
        ((CHUNK - 1) - (np.arange(N) % CHUNK)).astype(np.uint32)[None, :], (P, N)).copy()

    # i-pattern wrapped index constant per row-tile:
    # edge slot s = 480q + 16k + r  ->  (pt = 16q + r, k);  wrapped[r, 30q + k] holds i(s)=t*128+16q+r
    iwrap = np.zeros((NT, 16, 8 * KNN), np.uint16)
    for t in range(NT):
        for r in range(16):
            for q in range(8):
                for k in range(KNN):
                    iwrap[t, r, KNN * q + k] = t * P + 16 * q + r

    def w(name):
        return np.ascontiguousarray(inputs[name], np.float32)

    common = {
        "iota_lc": iota_lc,
        "iwrap": iwrap,
        "mlp_w0k0": w("mlp_w0")[:128],
        "mlp_w0k1": w("mlp_w0")[128:],
        "mlp_b0": np.ascontiguousarray(w("mlp_b0").reshape(8, 128).T),
        "mlp_w1r": np.ascontiguousarray(w("mlp_w1").reshape(8, 128, 256).transpose(1, 0, 2)),
        "mlp_b1": np.ascontiguousarray(w("mlp_b1").reshape(2, 128).T),
        "mlp_w2r": np.ascontiguousarray(w("mlp_w2").reshape(2, 128, 128).transpose(1, 0, 2)),
        "mlp_b2": w("mlp_b2").reshape(128, 1),
        "fin_w": w("fin_w"),
        "fin_brep": np.broadcast_to(w("fin_b")[None, :], (128, 3)).copy(),
    }
    for l, c in ((1, 3), (2, 64), (3, 64)):
        w0 = w(f"c{l}_w0")
        common[f"c{l}_w0a"] = np.ascontiguousarray(w0[:c])
        common[f"c{l}_w0b"] = np.ascontiguousarray(w0[c:])
        common[f"c{l}_w1"] = w(f"c{l}_w1")
        common[f"c{l}_b0"] = w(f"c{l}_b0").reshape(64, 1)
        common[f"c{l}_b1"] = w(f"c{l}_b1").reshape(64, 1)

    in_maps = []
    for core in range(NCORES):
        m = dict(common)
        m["posT"] = np.ascontiguousarray(
            pos[core * CPC:(core + 1) * CPC].transpose(0, 2, 1))
        in_maps.append(m)
    return in_maps


def kernel(**inputs):
    from concourse.bass_utils import run_bass_kernel_spmd

    key = "prog"
    if key not in _PROGRAM_CACHE:
        _PROGRAM_CACHE[key] = _build_program()
    nc = _PROGRAM_CACHE[key]

    in_maps = _host_inputs(inputs)
    res = run_bass_kernel_spmd(nc, in_maps, list(range(NCORES)))
    outs = [res.results[i]["out"] for i in range(NCORES)]
    return np.concatenate(outs, axis=0).astype(np.float32)


if __name__ == "__main__":
    rng = np.random.default_rng(0)
    fake = {"pos": rng.standard_normal((B, N, 3), np.float32)}
    # quick build-only check
    _build_program()
    print("program built ok")



# revision 5
# speedup vs baseline: 1.0564x; 1.0564x over previous
"""DGCNN (3x DynamicEdgeConv + point MLP) Trainium2 kernel, v2.

Self-contained: 2 point clouds per NeuronCore, SPMD on 8 cores
(data-parallel over the batch of 16). The two clouds on a core are
interleaved tile-by-tile so every engine always has independent work.

Per cloud, per edge-conv layer (feature-major layout X^T [C, N], bf16):
  S = X^T.T @ [2X ; -|x_j|^2]           (TensorE, bf16, fp32 PSUM)
  pack: packed = (S & ~0x7FF) | (2047-j)   one DVE scalar_tensor_tensor
        (index rides in the 11 mantissa LSBs; max order preserved)
  screen: top-8 per 128-chunk via MAX8 (16 ops) -> 128 candidates
  merge: 4 x (MAX8 + MATCH_REPLACE8) -> top-32 packed values
  extract: idx = (bits & 0x7FF) ^ 0x7FF  (one tensor_scalar)
  idx wrap: SBUF->DRAM->SBUF DMA bounce reshapes [128,30] -> 4 x [16,240]
  gather: GPSIMD indirect_copy (bf16), D = [x_j feats ; x_i feats]
  edge MLP on TensorE (h1 = relu(W@[xj;xi]+b0); h2 = w1@h1 written
  k-major to PSUM), ScalarE copies h2 -> SBUF bf16 point-major,
  one DVE tensor_reduce (max over 30 edges) + b1 add -> next features.
Then the pointwise MLP 192->1024->256->128->3 on TensorE (bf16).
"""
import sys
import numpy as np
import ml_dtypes

for _p in ("/opt/trn_rl_repo", "/root/.axon_site/_ro/trn_rl_repo"):
    if _p not in sys.path:
        sys.path.append(_p)

bfnp = ml_dtypes.bfloat16

B, N, KNN = 16, 2048, 30
NCORES = 8
CPC = B // NCORES          # clouds per core
P = 128
NT = N // P                # 16 row tiles per cloud
KSEL = 32
NEG = -3.0e38

_PROGRAM_CACHE = {}


def _build_program():
    import concourse.bass as bass
    import concourse.bacc as bacc
    import concourse.tile as tile
    from concourse import mybir
    from contextlib import ExitStack

    f32 = mybir.dt.float32
    bf16 = mybir.dt.bfloat16
    u32 = mybir.dt.uint32
    u16 = mybir.dt.uint16
    Alu = mybir.AluOpType
    Act = mybir.ActivationFunctionType

    nc = bacc.Bacc()

    def din(name, shape, dtype):
        return nc.declare_dram_parameter(name, list(shape), dtype, isOutput=False)

    posT_d = din("posT", [CPC, 3, N], bf16)
    iota_d = din("iota_c", [P, N], u32)
    maskp_d = din("maskp", [P, 1], u32)
    m7ff_d = din("m7ff", [P, 1], u32)
    iwrap_d = din("iwrap", [NT, 16, 8 * KNN], u16)
    conv_w = []
    for l in range(3):
        conv_w.append(dict(
            wh=din(f"wh{l}", [128, 64], bf16),
            w1=din(f"w1_{l}", [64, 64], bf16),
            b0=din(f"b0_{l}", [64, 1], f32),
            b1=din(f"b1_{l}", [64, 1], f32),
        ))
    mw0k0_d = din("mlp_w0k0", [128, 1024], bf16)
    mw0k1_d = din("mlp_w0k1", [64, 1024], bf16)
    mb0_d = din("mlp_b0", [128, 8], f32)
    mw1_d = din("mlp_w1r", [128, 8, 256], bf16)
    mb1_d = din("mlp_b1", [128, 2], f32)
    mw2_d = din("mlp_w2r", [128, 2, 128], bf16)
    mb2_d = din("mlp_b2", [128, 1], f32)
    finw_d = din("fin_w", [128, 3], bf16)
    finb_d = din("fin_brep", [128, 3], f32)
    out_d = nc.declare_dram_parameter("out", [CPC, N, 3], f32, isOutput=True)

    with tile.TileContext(nc) as tc, ExitStack() as CTX:
        persist = CTX.enter_context(tc.tile_pool(name="persist", bufs=1))
        cloudp = CTX.enter_context(tc.tile_pool(name="clouds", bufs=1))

        iota = persist.tile([P, N], u32)
        nc.sync.dma_start(iota, iota_d[:])
        maskp = persist.tile([P, 1], u32)
        nc.sync.dma_start(maskp, maskp_d[:])
        m7ff = persist.tile([P, 1], u32)
        nc.sync.dma_start(m7ff, m7ff_d[:])
        ones_col = persist.tile([64, 1], f32)
        nc.vector.memset(ones_col, 1.0)

        wh, w1, b0, b1 = [], [], [], []
        for l in range(3):
            t_ = persist.tile([128, 64], bf16, name=f"wh{l}")
            nc.sync.dma_start(t_, conv_w[l]["wh"][:])
            wh.append(t_)
            t_ = persist.tile([64, 64], bf16, name=f"w1_{l}")
            nc.sync.dma_start(t_, conv_w[l]["w1"][:])
            w1.append(t_)
            t_ = persist.tile([64, 1], f32, name=f"b0_{l}")
            nc.sync.dma_start(t_, conv_w[l]["b0"][:])
            b0.append(t_)
            t_ = persist.tile([64, 1], f32, name=f"b1_{l}")
            nc.sync.dma_start(t_, conv_w[l]["b1"][:])
            b1.append(t_)

        mw0k0 = persist.tile([128, 1024], bf16)
        nc.sync.dma_start(mw0k0, mw0k0_d[:])
        mw0k1 = persist.tile([64, 1024], bf16)
        nc.sync.dma_start(mw0k1, mw0k1_d[:])
        mb0 = persist.tile([128, 8], f32)
        nc.sync.dma_start(mb0, mb0_d[:])
        mw1 = persist.tile([128, 8, 256], bf16)
        nc.sync.dma_start(mw1, mw1_d[:])
        mb1 = persist.tile([128, 2], f32)
        nc.sync.dma_start(mb1, mb1_d[:])
        mw2 = persist.tile([128, 2, 128], bf16)
        nc.sync.dma_start(mw2, mw2_d[:])
        mb2 = persist.tile([128, 1], f32)
        nc.sync.dma_start(mb2, mb2_d[:])
        finw = persist.tile([128, 3], bf16)
        nc.sync.dma_start(finw, finw_d[:])
        finb = persist.tile([128, 3], f32)
        nc.sync.dma_start(finb, finb_d[:])

        # persistent index tiles: [0:64] j-rows (per layer), [64:128] i-rows
        it = [[persist.tile([P, 8 * KNN], u16, name=f"it{c}_{t}", tag="it", bufs=2 * NT)
               for t in range(NT)] for c in range(CPC)]

        # per-cloud feature tiles
        xt1, f1, f2, f3, D, x12 = [], [], [], [], [], []
        for c in range(CPC):
            xt1.append(cloudp.tile([34, N], bf16, name=f"xt1_{c}"))
            f1.append(cloudp.tile([66, N], bf16, name=f"f1_{c}"))
            f2.append(cloudp.tile([66, N], bf16, name=f"f2_{c}"))
            f3.append(cloudp.tile([64, N], bf16, name=f"f3_{c}"))
            D.append(cloudp.tile([P, N], bf16, name=f"D_{c}"))
            x12.append(cloudp.tile([P, N], bf16, name=f"x12_{c}"))
        xt_in = [[xt1[c], f1[c], f2[c]] for c in range(CPC)]   # per-layer S input
        xt_out = [[f1[c], f2[c], f3[c]] for c in range(CPC)]
        AUG = [32, 64, 64]                                     # ones-row partition

        with ExitStack() as conv_ctx:
            rp = conv_ctx.enter_context(tc.tile_pool(name="rp", bufs=1))
            sp_ps = conv_ctx.enter_context(tc.tile_pool(name="sp", bufs=2, space="PSUM"))
            e_ps = conv_ctx.enter_context(tc.tile_pool(name="eps", bufs=2, space="PSUM"))
            work = conv_ctx.enter_context(tc.tile_pool(name="wk", bufs=2))
            dramp = conv_ctx.enter_context(tc.tile_pool(name="drp", bufs=4, space="DRAM"))

            R = [[rp.tile([66, N], bf16, name=f"R{c}_{l}", tag=f"R{c}", bufs=2)
                  for l in range(3)] for c in range(CPC)]

            def build_R(cl, l):
                """R[cl][l] = [2*X ; -|x|^2] from the layer-l input features."""
                xt = xt_in[cl][l]
                C = 3 if l == 0 else 64
                aug = AUG[l]
                Rt = R[cl][l]
                if l == 0:
                    nc.vector.memset(Rt[0:34, :], 0.0)
                sq = work.tile([64, N], f32, tag="sq", name=f"sq{cl}_{l}")
                nc.scalar.activation(sq[0:C, :], xt[0:C, :], Act.Square)
                lo_t = work.tile([1, N], bf16, tag="lot", name=f"lot{cl}_{l}")
                for c4 in range(4):
                    sl = slice(c4 * 512, (c4 + 1) * 512)
                    x2ps = e_ps.tile([1, 512], f32, tag="h1p", name=f"x2ps{cl}_{l}_{c4}")
                    nc.tensor.matmul(x2ps, lhsT=ones_col[0:C, :], rhs=sq[0:C, sl],
                                     start=True, stop=True)
                    nc.scalar.activation(Rt[aug:aug + 1, sl], x2ps, Act.Copy, scale=-1.0)
                    nc.vector.scalar_tensor_tensor(
                        out=lo_t[:, sl], in0=x2ps, scalar=-1.0,
                        in1=Rt[aug:aug + 1, sl],
                        op0=Alu.mult, op1=Alu.subtract)
                nc.sync.dma_start(Rt[aug + 1:aug + 2, :], lo_t)
                nc.scalar.activation(Rt[0:C, :], xt[0:C, :], Act.Copy, scale=2.0)

            def build_D(cl, l):
                if l == 0:
                    nc.vector.memset(D[cl], 0.0)
                    nc.sync.dma_start(D[cl][0:3, :], posT_d[cl])
                    nc.sync.dma_start(D[cl][64:67, :], posT_d[cl])
                else:
                    f = xt_in[cl][l]
                    nc.sync.dma_start(D[cl][0:64, :], f[0:64, :])
                    nc.sync.dma_start(D[cl][64:128, :], f[0:64, :])

            # ---- init: layer-0 inputs ----
            for cl in range(CPC):
                nc.vector.memset(xt1[cl], 0.0)
                nc.sync.dma_start(xt1[cl][0:3, :], posT_d[cl])
                nc.vector.memset(xt1[cl][32:34, :], 1.0)
                nc.vector.memset(f1[cl][64:66, :], 1.0)
                nc.vector.memset(f2[cl][64:66, :], 1.0)
                build_D(cl, 0)
                build_R(cl, 0)

            packed_t, dscr_t = {}, {}

            def stage_a(cl, l, t):
                xt = xt_in[cl][l]
                caug = AUG[l] + 2
                lhsT = xt[0:caug, t * P:(t + 1) * P]
                pk = work.tile([P, N], f32, tag="packed", name=f"pk{cl}")
                for h in range(2):
                    sp = sp_ps.tile([P, 1024], f32, tag="spsum", name=f"sp{cl}_{h}")
                    for c2 in range(2):
                        cc = 2 * h + c2
                        nc.tensor.matmul(sp[:, c2 * 512:(c2 + 1) * 512],
                                         lhsT=lhsT,
                                         rhs=R[cl][l][0:caug, cc * 512:(cc + 1) * 512],
                                         start=True, stop=True)
                    nc.vector.scalar_tensor_tensor(
                        out=pk[:, h * 1024:(h + 1) * 1024].bitcast(u32),
                        in0=sp.bitcast(u32),
                        scalar=maskp,
                        in1=iota[:, h * 1024:(h + 1) * 1024],
                        op0=Alu.bitwise_and,
                        op1=Alu.bitwise_or,
                    )
                packed_t[(cl, t)] = pk

            def stage_b(cl, l, t):
                pk = packed_t.pop((cl, t))
                cand = work.tile([P, 128], f32, tag="cand")
                for c in range(16):
                    nc.vector.max(out=cand[:, c * 8:(c + 1) * 8],
                                  in_=pk[:, c * 128:(c + 1) * 128])
                topv = work.tile([P, KSEL], f32, tag="topv")
                cscr = work.tile([P, 128], f32, tag="cscr")
                cur = cand
                for r in range(KSEL // 8):
                    tv = topv[:, r * 8:(r + 1) * 8]
                    nc.vector.max(out=tv, in_=cur)
                    if r < KSEL // 8 - 1:
                        nxt = cscr if cur is cand else cand
                        nc.vector.match_replace(out=nxt, in_to_replace=tv,
                                                in_values=cur, imm_value=NEG)
                        cur = nxt
                idxs = work.tile([P, KSEL], u32, tag="idxs")
                nc.vector.tensor_scalar(out=idxs, in0=topv.bitcast(u32),
                                        scalar1=m7ff, scalar2=m7ff,
                                        op0=Alu.bitwise_and, op1=Alu.bitwise_xor)
                idx16 = work.tile([P, KSEL], u16, tag="idx16")
                nc.vector.tensor_copy(idx16, idxs)
                ds = dramp.tile([P, KNN], u16, tag="dscr")
                nc.sync.dma_start(ds[:], idx16[:, 0:KNN])
                dscr_t[(cl, t)] = ds

            def stage_c(cl, l, t):
                ds = dscr_t.pop((cl, t))
                itt = it[cl][t]
                src = ds.rearrange("(q r) k -> r q k", q=8)
                for g in range(4):
                    nc.sync.dma_start(itt[16 * g:16 * (g + 1), :], src)
                if l == 0:   # preload constant i-rows (persist across layers)
                    for g in range(4):
                        nc.sync.dma_start(itt[64 + 16 * g:80 + 16 * g, :], iwrap_d[t])
                G = work.tile([P, P * KNN], bf16, tag="G")
                for g in range(4):
                    nc.gpsimd.indirect_copy(
                        out=G[:, 960 * g:960 * (g + 1)], data=D[cl],
                        idxs=itt[:, 60 * g:60 * (g + 1)],
                        i_know_ap_gather_is_preferred=True)
                h2sb = work.tile([64, P * KNN], bf16, tag="h2sb", name=f"h2sb{cl}")
                for q in range(8):
                    gsl = G[:, q * 480:(q + 1) * 480]
                    h1p = e_ps.tile([64, 480], f32, tag="h1p")
                    nc.tensor.matmul(h1p, lhsT=wh[l], rhs=gsl, start=True, stop=True)
                    h1s = work.tile([64, 480], bf16, tag="h1s")
                    nc.scalar.activation(h1s, h1p, Act.Relu, bias=b0[l])
                    h2p = e_ps.tile([64, 16, KNN], f32, tag="h2p")
                    nc.tensor.matmul(h2p.transpose([0, 2, 1]), lhsT=w1[l], rhs=h1s,
                                     start=True, stop=True)
                    nc.scalar.activation(h2sb[:, q * 480:(q + 1) * 480], h2p, Act.Copy)
                red = work.tile([64, P], bf16, tag="red")
                nc.vector.tensor_reduce(out=red,
                                        in_=h2sb.rearrange("p (x k) -> p x k", k=KNN),
                                        axis=mybir.AxisListType.X, op=Alu.max)
                nc.vector.tensor_scalar_add(
                    xt_out[cl][l][0:64, t * P:(t + 1) * P], red, b1[l])

            for l in range(3):
                for t in range(NT + 2):
                    for cl in range(CPC):
                        if t < NT:
                            stage_a(cl, l, t)
                    for cl in range(CPC):
                        if 1 <= t <= NT:
                            stage_b(cl, l, t - 1)
                    for cl in range(CPC):
                        if t >= 2:
                            stage_c(cl, l, t - 2)
                if l < 2:
                    for cl in range(CPC):
                        build_R(cl, l + 1)
                        build_D(cl, l + 1)

            # stack [x1; x2] for the MLP while conv pools are still open
            for cl in range(CPC):
                nc.sync.dma_start(x12[cl][0:64, :], f1[cl][0:64, :])
                nc.sync.dma_start(x12[cl][64:128, :], f2[cl][0:64, :])

        # ---------------- pointwise MLP ----------------
        with ExitStack() as mctx:
            mp = mctx.enter_context(tc.tile_pool(name="mlp", bufs=2))
            mps = mctx.enter_context(tc.tile_pool(name="mlpp", bufs=4, space="PSUM"))
            NCHK = 512
            for cl in range(CPC):
                for nchk in range(N // NCHK):
                    csl = slice(nchk * NCHK, (nchk + 1) * NCHK)
                    h1m = mp.tile([128, 8, NCHK], bf16, tag="h1m")
                    for m in range(8):
                        msl = slice(m * 128, (m + 1) * 128)
                        hp = mps.tile([128, NCHK], f32, tag="hp")
                        nc.tensor.matmul(hp, lhsT=mw0k0[:, msl], rhs=x12[cl][:, csl],
                                         start=True, stop=False)
                        nc.tensor.matmul(hp, lhsT=mw0k1[:, msl], rhs=f3[cl][:, csl],
                                         start=False, stop=True)
                        nc.scalar.activation(h1m[:, m, :], hp, Act.Relu,
                                             bias=mb0[:, m:m + 1])
                    h2m = mp.tile([128, 2, NCHK], bf16, tag="h2m")
                    for m in range(2):
                        hp = mps.tile([128, NCHK], f32, tag="hp")
                        for s in range(8):
                            nc.tensor.matmul(hp, lhsT=mw1[:, s, m * 128:(m + 1) * 128],
                                             rhs=h1m[:, s, :],
                                             start=(s == 0), stop=(s == 7))
                        nc.scalar.activation(h2m[:, m, :], hp, Act.Relu,
                                             bias=mb1[:, m:m + 1])
                    hp3 = mps.tile([128, NCHK], f32, tag="hp")
                    for s in range(2):
                        nc.tensor.matmul(hp3, lhsT=mw2[:, s, :], rhs=h2m[:, s, :],
                                         start=(s == 0), stop=(s == 1))
                    h3m = mp.tile([128, NCHK], bf16, tag="h3m")
                    nc.vector.tensor_scalar_add(h3m, hp3, mb2)
                    for pchk in range(NCHK // 128):
                        fp = mps.tile([128, 3], f32, tag="fp")
                        nc.tensor.matmul(fp, lhsT=h3m[:, pchk * 128:(pchk + 1) * 128],
                                         rhs=finw, start=True, stop=True)
                        fo = mp.tile([128, 3], f32, tag="fo")
                        nc.vector.tensor_tensor(out=fo, in0=fp, in1=finb, op=Alu.add)
                        r0 = nchk * NCHK + pchk * 128
                        nc.sync.dma_start(out_d[cl, r0:r0 + 128, :], fo)
    nc.compile()
    return nc


def _host_inputs(inputs):
    """Build the per-core input maps (pure layout/precision prep)."""
    pos = np.ascontiguousarray(inputs["pos"], np.float32)

    iota = np.broadcast_to((2047 - np.arange(N)).astype(np.uint32)[None, :],
                           (P, N)).copy()
    maskp = np.full((P, 1), 0xFFFFF800, np.uint32)
    m7ff = np.full((P, 1), 0x7FF, np.uint32)

    iwrap = np.zeros((NT, 16, 8 * KNN), np.uint16)
    c_ = np.arange(8 * KNN)
    for t in range(NT):
        for u in range(16):
            iwrap[t, u, :] = t * P + 16 * (c_ // KNN) + u

    def w(name):
        return np.ascontiguousarray(inputs[name], np.float32)

    common = {
        "iota_c": iota, "maskp": maskp, "m7ff": m7ff, "iwrap": iwrap,
        "mlp_w0k0": w("mlp_w0")[:128].astype(bfnp),
        "mlp_w0k1": w("mlp_w0")[128:].astype(bfnp),
        "mlp_b0": np.ascontiguousarray(w("mlp_b0").reshape(8, 128).T),
        "mlp_w1r": np.ascontiguousarray(
            w("mlp_w1").reshape(8, 128, 256).transpose(1, 0, 2)).astype(bfnp),
        "mlp_b1": np.ascontiguousarray(w("mlp_b1").reshape(2, 128).T),
        "mlp_w2r": np.ascontiguousarray(
            w("mlp_w2").reshape(2, 128, 128).transpose(1, 0, 2)).astype(bfnp),
        "mlp_b2": w("mlp_b2").reshape(128, 1),
        "fin_w": w("fin_w").astype(bfnp),
        "fin_brep": np.broadcast_to(w("fin_b")[None, :], (128, 3)).copy(),
    }
    for l, (nm, c) in enumerate((("c1", 3), ("c2", 64), ("c3", 64))):
        w0 = w(f"{nm}_w0")
        w0a, w0b = w0[:c], w0[c:]
        whl = np.zeros((128, 64), np.float32)
        whl[0:c] = w0b
        whl[64:64 + c] = w0a - w0b
        common[f"wh{l}"] = whl.astype(bfnp)
        common[f"w1_{l}"] = w(f"{nm}_w1").astype(bfnp)
        common[f"b0_{l}"] = w(f"{nm}_b0").reshape(64, 1)
        common[f"b1_{l}"] = w(f"{nm}_b1").reshape(64, 1)

    in_maps = []
    for core in range(NCORES):
        m = dict(common)
        m["posT"] = np.ascontiguousarray(
            pos[core * CPC:(core + 1) * CPC].transpose(0, 2, 1)).astype(bfnp)
        in_maps.append(m)
    return in_maps


def kernel(**inputs):
    from concourse.bass_utils import run_bass_kernel_spmd

    key = "prog"
    if key not in _PROGRAM_CACHE:
        _PROGRAM_CACHE[key] = _build_program()
    nc = _PROGRAM_CACHE[key]

    in_maps = _host_inputs(inputs)
    res = run_bass_kernel_spmd(nc, in_maps, list(range(NCORES)))
    outs = [res.results[i]["out"] for i in range(NCORES)]
    return np.concatenate(outs, axis=0).astype(np.float32)


if __name__ == "__main__":
    _build_program()
    print("program built ok")
